# revision 5
# baseline (speedup 1.0000x reference)
"""DDALoss Trainium2 kernel (8 NeuronCores, class-sharded softmax).

Math (algebraically identical to the reference):
  g[n,c]     = 2*feat[n]@centers[c] - ||centers[c]||^2          (logits shifted
               by the row-constant ||feat[n]||^2, which cancels in softmax)
  lse[n]     = log(sum_c exp(g[n,c]))
  glab[n]    = g[n, label[n]]
  nll_sum    = sum_n (lse[n] - glab[n])
  S1         = sum(feat^2)
  centerloss = (S1 - sum_n glab[n]) / (2N)
  ddaloss    = nll_sum / (2N^2)
  loss       = LAMB*centerloss + GAMMA*ddaloss

Sharding: classes are split 8 ways (1280 padded classes per core); every core
sees all 4096 batch rows.  vs batch-sharding this cuts per-core DMA from
~18MB to ~6MB (no full 10240x512 bf16 centers stream per core) and the csq
DVE work by 8x.  Per-row partial sums of exp are combined with a 16KB
AllReduce; the label/gather path stays batch-sharded (rows i*512..(i+1)*512
on core i) so scalar partials just sum on the host.

Per-core schedule:
  - everything SBUF-resident up front: featT fp8 [128,4,4096] (weights),
    centers-shard^T fp8 [128,4,1280] (moving), centers-shard bf16 natural
    [128,10,512] (for csq only).
  - csq chain: 10 DVE TENSOR_TENSOR_REDUCE squares -> csqn[:,j], PE transpose
    -> fp8 cast -> DRAM roundtrip -> csqrow [1,2,1280] fp8 (plane 1 zeros).
  - main loop over 32 batch tiles: psum[n128, c1280] accumulates 6 fp8
    DoubleRow matmuls (K=512) plus 3 K=2 DoubleRow "ones x (-csq*FS*CS/2)"
    bias matmuls; ACT exp(scale*psum) with accum_out -> partial sumexp col.
  - AllReduce [128,32] partial sumexp across the 8 cores, ln -> lse, then
    sum-reduce. Every core emits the identical full lse_sum (host divides
    by NCORES); glab/S1 partials are per-core as in the batch-sharded path.
  - output: [1,3] partials (lse_sum, glab_sum, S1); final combine on host.
"""

import sys

sys.path.insert(0, "/opt/trn_rl_repo")

import numpy as np
import ml_dtypes

from contextlib import ExitStack

import concourse.bass as bass
import concourse.bacc as bacc
import concourse.tile as tile
from concourse import mybir

# Problem constants (hardcoded per harness contract)
N = 4096
D = 512
C = 10000
CP = 10240  # classes padded to 128*80
NCORES = 8
CPC = CP // NCORES  # 1280 classes per core
NPC = N // NCORES  # 512 label rows per core
NT = N // 128  # 32 batch tiles per core (all rows)
NTL = NPC // 128  # 4 label tiles per core
KT = D // 128  # 4 contraction blocks
CSUB = [512, 512, 256]  # class sub-chunks within the 1280-wide psum tile
COFF = [0, 512, 1024]

LAMB = 0.01
GAMMA = 3.0

BF16 = mybir.dt.bfloat16
FP8 = mybir.dt.float8e4
F32 = mybir.dt.float32
I32 = mybir.dt.int32

# fp8 scaling: feat*FS and centers*CS on host keep e4m3 values in the normal
# range; psum then holds FS*CS*cross, the bias row holds -(FS*CS/2)*csq, and
# ACT's exp scale of 2/(FS*CS) restores exp(2*cross - csq).
FS = 8.0
CS = 16.0
# value written into padded center rows: csq_pad = 512*PADV^2 = 6.77 so the
# fp8 bias -(FS*CS/2)*csq_pad = -433 stays inside e4m3 range (no NaN) while
# exp(-6.77)*240 pad classes contribute only ~2e-5 of a typical row sum.
PADV = 0.115

_CACHE = {}


def _ttr(nc, out, in0, in1, accum_out, init, scale=1.0):
    """accum_out = init + sum_free(in0 * in1 * scale); out = elementwise scratch."""
    from concourse.dve_ops import TENSOR_TENSOR_REDUCE

    nc.vector._custom_dve(
        TENSOR_TENSOR_REDUCE,
        out=out,
        in0=in0,
        in1=in1,
        s0=init,
        s1=scale,
        accum_out=accum_out,
    )


def _build():
    nc = bacc.Bacc(
        "TRN2", target_bir_lowering=False, debug=False, num_devices=NCORES
    )

    # Per-core external inputs
    ftT = nc.dram_tensor("ftt", [D, N], FP8, kind="ExternalInput")  # full feat^T
    fnat = nc.dram_tensor("fnat", [NPC, D], F32, kind="ExternalInput")  # feat rows
    lab = nc.dram_tensor("lab", [NPC, 1], I32, kind="ExternalInput")
    cT = nc.dram_tensor("ct", [D, CPC], FP8, kind="ExternalInput")  # centers shard^T
    cnat = nc.dram_tensor("cnat", [CPC, D], BF16, kind="ExternalInput")  # shard natural
    cfull = nc.dram_tensor("cfull", [C, D], F32, kind="ExternalInput")  # for gather
    out = nc.dram_tensor("out", [1, 3], F32, kind="ExternalOutput")
    out2 = nc.dram_tensor("out2", [128, NT], F32, kind="ExternalOutput")
    csq_dram = nc.dram_tensor("csq_scratch", [CPC // 128, 128], FP8, kind="Internal")

    with tile.TileContext(nc) as tc, ExitStack() as ctx:
        const = ctx.enter_context(tc.tile_pool(name="const", bufs=1))
        small = ctx.enter_context(tc.tile_pool(name="small", bufs=2))
        scrp = ctx.enter_context(tc.tile_pool(name="scrp", bufs=2))
        expp = ctx.enter_context(tc.tile_pool(name="expp", bufs=2))
        ps_small = ctx.enter_context(tc.tile_pool(name="ps_small", bufs=1, space="PSUM"))

        # ---- constants / persistent tiles ----
        ones_f = const.tile([128, 1], F32)
        nc.vector.memset(ones_f, 1.0)
        ones_f8 = const.tile([1, 2, 128], FP8)
        nc.vector.memset(ones_f8[:1, 0, :], 1.0)
        nc.vector.memset(ones_f8[:1, 1, :], 0.0)
        ident = const.tile([128, 128], F32, tag="ident")
        from concourse.masks import make_identity

        make_identity(nc, ident)

        # all SBUF-resident operands, loaded once
        ct_t = const.tile([128, KT, CPC], FP8, tag="ct_t")
        cT_r = cT.ap().rearrange("(k p) c -> p k c", p=128)
        nc.sync.dma_start(out=ct_t, in_=cT_r)

        cn = const.tile([128, CPC // 128, D], BF16, tag="cn")
        cnat_r = cnat.ap().rearrange("(x p) d -> p x d", p=128)
        nc.sync.dma_start(out=cn, in_=cnat_r)

        ft = const.tile([128, KT, N], FP8, tag="ft")
        ftT_r = ftT.ap().rearrange("(k p) n -> p k n", p=128)
        ft_dmas = []
        for i in range(4):
            ft_dmas.append(
                nc.sync.dma_start(
                    out=ft[:, :, i * 1024 : (i + 1) * 1024],
                    in_=ftT_r[:, :, i * 1024 : (i + 1) * 1024],
                )
            )

        csqn = const.tile([128, CPC // 128], F32, tag="csqn")  # -(FS*CS/2)*csq
        csqrow = const.tile([1, 2, CPC], FP8, tag="csqrow")  # plane0 bias, plane1 0
        nc.vector.memset(csqrow[:1, 1, :], 0.0)
        accg = const.tile([128, NT], F32, tag="accg")  # per-nt partial sumexp
        cl4 = const.tile([128, NTL], F32, tag="cl4")
        cq4 = const.tile([128, NTL], F32, tag="cq4")
        fsq4 = const.tile([128, NTL], F32, tag="fsq4")
        fin3 = const.tile([128, 3], F32, tag="fin3")

        # ---- csq chain: cn -> csqn -> (transpose+cast) -> csqrow ----
        for j in range(CPC // 128):
            scr = scrp.tile([128, D], BF16, tag="csq_scr")
            _ttr(
                nc,
                scr,
                cn[:, j, :],
                cn[:, j, :],
                csqn[:, j : j + 1],
                0.0,
                scale=-(FS * CS / 2.0),
            )
        tp = ps_small.tile([CPC // 128, 128], F32, tag="tp")
        nc.tensor.transpose(out=tp, in_=csqn[:, :], identity=ident)
        tp_f8 = small.tile([CPC // 128, 128], FP8, tag="tp_f8")
        nc.vector.tensor_copy(tp_f8, tp)
        nc.sync.dma_start(out=csq_dram.ap(), in_=tp_f8)
        nc.sync.dma_start(
            out=csqrow[:1, 0, :],
            in_=bass.AP(tensor=csq_dram, offset=0, ap=[[0, 1], [1, CPC]]),
        )

        # ---- main loop over batch tiles ----
        with tc.tile_pool(name="ps_g", bufs=2, space="PSUM") as ps_g:
            for nt in range(NT):
                g = ps_g.tile([128, 1536], F32, tag="g")
                for k in range(0, KT, 2):
                    for s in range(3):
                        nc.tensor.matmul(
                            out=g[:, COFF[s] : COFF[s] + CSUB[s]],
                            lhsT=ft[:, k : k + 2, nt * 128 : (nt + 1) * 128],
                            rhs=ct_t[:, k : k + 2, COFF[s] : COFF[s] + CSUB[s]],
                            start=(k == 0),
                            stop=False,
                            perf_mode=mybir.MatmulPerfMode.DoubleRow,
                        )
                for s in range(3):
                    nc.tensor.matmul(
                        out=g[:, COFF[s] : COFF[s] + CSUB[s]],
                        lhsT=ones_f8,
                        rhs=csqrow[:1, :, COFF[s] : COFF[s] + CSUB[s]],
                        start=False,
                        stop=True,
                        perf_mode=mybir.MatmulPerfMode.DoubleRow,
                    )
                scr_e = expp.tile([128, CPC], BF16, tag="scr_e")
                nc.scalar.activation(
                    scr_e,
                    g[:, :CPC],
                    mybir.ActivationFunctionType.Exp,
                    scale=2.0 / (FS * CS),
                    accum_out=accg[:, nt : nt + 1],
                )

        # ---- label path (independent; gather from full centers in DRAM) ----
        for nt in range(NTL):
            labt = small.tile([128, 1], I32, tag="labt")
            d1 = nc.sync.dma_start(
                out=labt, in_=lab.ap()[nt * 128 : (nt + 1) * 128, :]
            )
            tile.add_dep_helper(d1.ins, ft_dmas[3].ins, True, "defer label path")
            crows = small.tile([128, D], F32, tag="crows")
            nc.gpsimd.indirect_dma_start(
                out=crows,
                out_offset=None,
                in_=cfull.ap(),
                in_offset=bass.IndirectOffsetOnAxis(ap=labt[:, :1], axis=0),
            )
            fnt = small.tile([128, D], F32, tag="fnt")
            d2 = nc.sync.dma_start(
                out=fnt, in_=fnat.ap()[nt * 128 : (nt + 1) * 128, :]
            )
            tile.add_dep_helper(d2.ins, ft_dmas[3].ins, True, "defer label path")
            scr1 = scrp.tile([128, D], F32, tag="lab_scr")
            _ttr(nc, scr1, fnt, crows, cl4[:, nt : nt + 1], 0.0)
            scr2 = scrp.tile([128, D], F32, tag="lab_scr")
            _ttr(nc, scr2, crows, crows, cq4[:, nt : nt + 1], 0.0)
            scr3 = scrp.tile([128, D], F32, tag="lab_scr")
            _ttr(nc, scr3, fnt, fnt, fsq4[:, nt : nt + 1], 0.0)

        # ---- combine partial sumexp across cores (DEBUG: host-side) ----
        nc.sync.dma_start(out=out2.ap(), in_=accg)

        # ---- finals ----
        nc.vector.memset(fin3[:, 0:1], 0.0)
        glab4 = small.tile([128, NTL], F32, tag="glab4")
        nc.vector.tensor_scalar_mul(glab4, cl4, 2.0)
        nc.vector.tensor_sub(glab4, glab4, cq4)
        nc.vector.reduce_sum(fin3[:, 1:2], glab4, axis=mybir.AxisListType.X)
        nc.vector.reduce_sum(fin3[:, 2:3], fsq4, axis=mybir.AxisListType.X)
        fin_ps = ps_small.tile([1, 3], F32, tag="fin_ps")
        nc.tensor.matmul(out=fin_ps, lhsT=ones_f, rhs=fin3, start=True, stop=True)
        out_sb = small.tile([1, 3], F32, tag="out_sb")
        nc.scalar.copy(out_sb, fin_ps)
        nc.sync.dma_start(out=out.ap(), in_=out_sb)

    nc.compile()
    return nc


def _get_nc():
    if "nc" not in _CACHE:
        _CACHE["nc"] = _build()
    return _CACHE["nc"]


def make_in_maps(feat, label, centers):
    feat = np.ascontiguousarray(np.asarray(feat, dtype=np.float32))
    centers = np.ascontiguousarray(np.asarray(centers, dtype=np.float32))
    label = np.ascontiguousarray(np.asarray(label).astype(np.int32).reshape(N, 1))

    bf = ml_dtypes.bfloat16
    f8 = ml_dtypes.float8_e4m3
    cT_pad = np.zeros((D, CP), dtype=f8)
    cT_pad[:, :C] = (centers.T * CS).astype(f8)
    cnat_pad = np.full((CP, D), PADV, dtype=bf)
    cnat_pad[:C, :] = centers.astype(bf)
    featT = np.ascontiguousarray((feat.T * FS).astype(f8))  # [D, N]

    in_maps = []
    for i in range(NCORES):
        sl = slice(i * NPC, (i + 1) * NPC)
        cs = slice(i * CPC, (i + 1) * CPC)
        in_maps.append(
            {
                "ftt": featT,
                "fnat": np.ascontiguousarray(feat[sl]),
                "lab": np.ascontiguousarray(label[sl]),
                "ct": np.ascontiguousarray(cT_pad[:, cs]),
                "cnat": np.ascontiguousarray(cnat_pad[cs]),
                "cfull": centers,
            }
        )
    return in_maps


def combine(parts, accgs):
    parts = np.asarray(parts, dtype=np.float64)
    sumexp = np.zeros((128, NT), dtype=np.float64)
    for a in accgs:
        sumexp += np.asarray(a, dtype=np.float64)
    lse_sum = float(np.log(sumexp).sum())  # DEBUG ONLY: host-side ln
    glab_sum = parts[:, 1].sum()
    s1 = parts[:, 2].sum()
    nll_sum = lse_sum - glab_sum
    centerloss = (s1 - glab_sum) / (2.0 * N)
    ddaloss = nll_sum / (2.0 * N * N)
    loss = LAMB * centerloss + GAMMA * ddaloss
    return loss, centerloss, ddaloss


def kernel(feat, label, centers):
    from concourse.bass_utils import run_bass_kernel_spmd

    in_maps = make_in_maps(feat, label, centers)
    nc = _get_nc()
    res = run_bass_kernel_spmd(nc, in_maps, core_ids=list(range(NCORES)))
    parts = [r["out"].reshape(3) for r in res.results]
    accgs = [r["out2"] for r in res.results]
    loss, centerloss, ddaloss = combine(parts, accgs)
    return (
        np.float32(loss),
        np.float32(centerloss),
        np.float32(ddaloss),
    )


# revision 10
# speedup vs baseline: 1.0207x; 1.0207x over previous
"""DDALoss Trainium2 kernel (8 NeuronCores, class-sharded softmax).

Math (algebraically identical to the reference):
  g[n,c]     = 2*feat[n]@centers[c] - ||centers[c]||^2          (logits shifted
               by the row-constant ||feat[n]||^2, which cancels in softmax)
  lse[n]     = log(sum_c exp(g[n,c]))
  glab[n]    = g[n, label[n]]
  nll_sum    = sum_n (lse[n] - glab[n])
  S1         = sum(feat^2)
  centerloss = (S1 - sum_n glab[n]) / (2N)
  ddaloss    = nll_sum / (2N^2)
  loss       = LAMB*centerloss + GAMMA*ddaloss

Sharding: classes are split 8 ways (1280 padded classes per core); every core
sees all 4096 batch rows.  vs batch-sharding this cuts per-core DMA from
~18MB to ~6MB (no full 10240x512 bf16 centers stream per core) and the csq
DVE work by 8x.  Per-row partial sums of exp are combined with a 16KB
AllReduce; the label/gather path stays batch-sharded (rows i*512..(i+1)*512
on core i) so scalar partials just sum on the host.

Per-core schedule:
  - everything SBUF-resident up front: featT fp8 [128,4,4096] (weights),
    centers-shard^T fp8 [128,4,1280] (moving), centers-shard bf16 natural
    [128,10,512] (for csq only).
  - csq chain: 10 DVE TENSOR_TENSOR_REDUCE squares -> csqn[:,j], PE transpose
    -> fp8 cast -> DRAM roundtrip -> csqrow [1,2,1280] fp8 (plane 1 zeros).
  - main loop over 32 batch tiles: psum[n128, c1280] accumulates 6 fp8
    DoubleRow matmuls (K=512) plus 3 K=2 DoubleRow "ones x (-csq*FS*CS/2)"
    bias matmuls; ACT exp(scale*psum) with accum_out -> partial sumexp col.
  - AllReduce [128,32] partial sumexp across the 8 cores, ln -> lse, then
    sum-reduce. Every core emits the identical full lse_sum (host divides
    by NCORES); glab/S1 partials are per-core as in the batch-sharded path.
  - output: [1,3] partials (lse_sum, glab_sum, S1); final combine on host.
"""

import sys

sys.path.insert(0, "/opt/trn_rl_repo")

import numpy as np
import ml_dtypes

from contextlib import ExitStack

import concourse.bass as bass
import concourse.bacc as bacc
import concourse.tile as tile
from concourse import mybir

# Problem constants (hardcoded per harness contract)
N = 4096
D = 512
C = 10000
CP = 10240  # classes padded to 128*80
NCORES = 8
CPC = CP // NCORES  # 1280 classes per core
NPC = N // NCORES  # 512 label rows per core
NT = N // 128  # 32 batch tiles per core (all rows)
NTL = NPC // 128  # 4 label tiles per core
KT = D // 128  # 4 contraction blocks
CSUB = [512, 512, 256]  # class sub-chunks within the 1280-wide psum tile
COFF = [0, 512, 1024]

LAMB = 0.01
GAMMA = 3.0

BF16 = mybir.dt.bfloat16
FP8 = mybir.dt.float8e4
F32 = mybir.dt.float32
I32 = mybir.dt.int32

# fp8 scaling: feat*FS and centers*CS on host keep e4m3 values in the normal
# range; psum then holds FS*CS*cross, the bias row holds -(FS*CS/2)*csq, and
# ACT's exp scale of 2/(FS*CS) restores exp(2*cross - csq).
FS = 8.0
CS = 16.0
# value written into padded center rows: csq_pad = 512*PADV^2 = 6.77 so the
# fp8 bias -(FS*CS/2)*csq_pad = -433 stays inside e4m3 range (no NaN) while
# exp(-6.77)*240 pad classes contribute only ~2e-5 of a typical row sum.
PADV = 0.115

_CACHE = {}


def _ttr(nc, out, in0, in1, accum_out, init, scale=1.0):
    """accum_out = init + sum_free(in0 * in1 * scale); out = elementwise scratch."""
    from concourse.dve_ops import TENSOR_TENSOR_REDUCE

    nc.vector._custom_dve(
        TENSOR_TENSOR_REDUCE,
        out=out,
        in0=in0,
        in1=in1,
        s0=init,
        s1=scale,
        accum_out=accum_out,
    )


def _build():
    nc = bacc.Bacc(
        "TRN2", target_bir_lowering=False, debug=False, num_devices=NCORES
    )

    # Per-core external inputs
    ftT = nc.dram_tensor("ftt", [D, N], FP8, kind="ExternalInput")  # full feat^T
    fnat = nc.dram_tensor("fnat", [NPC, D], F32, kind="ExternalInput")  # feat rows
    lab = nc.dram_tensor("lab", [NPC, 1], I32, kind="ExternalInput")
    cT = nc.dram_tensor("ct", [D, CPC], FP8, kind="ExternalInput")  # centers shard^T
    cnat = nc.dram_tensor("cnat", [CPC, D], BF16, kind="ExternalInput")  # shard natural
    cfull = nc.dram_tensor("cfull", [C, D], F32, kind="ExternalInput")  # for gather
    out = nc.dram_tensor("out", [1, 3], F32, kind="ExternalOutput")
    out2 = nc.dram_tensor("out2", [128, NT], F32, kind="ExternalOutput")
    csq_dram = nc.dram_tensor("csq_scratch", [CPC // 128, 128], BF16, kind="Internal")

    with tile.TileContext(nc) as tc, ExitStack() as ctx:
        const = ctx.enter_context(tc.tile_pool(name="const", bufs=1))
        small = ctx.enter_context(tc.tile_pool(name="small", bufs=2))
        scrp = ctx.enter_context(tc.tile_pool(name="scrp", bufs=2))
        expp = ctx.enter_context(tc.tile_pool(name="expp", bufs=2))
        ps_small = ctx.enter_context(tc.tile_pool(name="ps_small", bufs=1, space="PSUM"))

        # ---- constants / persistent tiles ----
        ones_f = const.tile([128, 1], F32)
        nc.vector.memset(ones_f, 1.0)
        ones_b = const.tile([1, 128], BF16)
        nc.vector.memset(ones_b, 1.0)
        ident = const.tile([128, 128], F32, tag="ident")
        from concourse.masks import make_identity

        make_identity(nc, ident)

        # all SBUF-resident operands, loaded once
        ct_t = const.tile([128, KT, CPC], FP8, tag="ct_t")
        cT_r = cT.ap().rearrange("(k p) c -> p k c", p=128)
        nc.sync.dma_start(out=ct_t, in_=cT_r)

        cn = const.tile([128, CPC // 128, D], BF16, tag="cn")
        cnat_r = cnat.ap().rearrange("(x p) d -> p x d", p=128)
        nc.sync.dma_start(out=cn, in_=cnat_r)

        ft = const.tile([128, KT, N], FP8, tag="ft")
        ftT_r = ftT.ap().rearrange("(k p) n -> p k n", p=128)
        ft_dmas = []
        for i in range(4):
            ft_dmas.append(
                nc.sync.dma_start(
                    out=ft[:, :, i * 1024 : (i + 1) * 1024],
                    in_=ftT_r[:, :, i * 1024 : (i + 1) * 1024],
                )
            )

        csqn = const.tile([128, CPC // 128], F32, tag="csqn")  # -(FS*CS/2)*csq
        csqrow = const.tile([1, CPC], BF16, tag="csqrow")  # bias row, class order
        accg = const.tile([128, NT], F32, tag="accg")  # per-nt partial sumexp
        cl4 = const.tile([128, NTL], F32, tag="cl4")
        cq4 = const.tile([128, NTL], F32, tag="cq4")
        fsq4 = const.tile([128, NTL], F32, tag="fsq4")
        fin3 = const.tile([128, 3], F32, tag="fin3")

        # ---- csq chain: cn -> csqn -> (transpose+cast) -> csqrow ----
        for j in range(CPC // 128):
            scr = scrp.tile([128, D], BF16, tag="csq_scr")
            _ttr(
                nc,
                scr,
                cn[:, j, :],
                cn[:, j, :],
                csqn[:, j : j + 1],
                0.0,
                scale=-(FS * CS / 2.0),
            )
        tp = ps_small.tile([CPC // 128, 128], F32, tag="tp")
        nc.tensor.transpose(out=tp, in_=csqn[:, :], identity=ident)
        tp_b = small.tile([CPC // 128, 128], BF16, tag="tp_b")
        nc.vector.tensor_copy(tp_b, tp)
        nc.sync.dma_start(out=csq_dram.ap(), in_=tp_b)
        nc.sync.dma_start(
            out=csqrow,
            in_=bass.AP(tensor=csq_dram, offset=0, ap=[[0, 1], [1, CPC]]),
        )

        # ---- main loop over batch tiles ----
        with tc.tile_pool(name="ps_g", bufs=2, space="PSUM") as ps_g:
            for nt in range(NT):
                g = ps_g.tile([128, 1536], F32, tag="g")
                for k in range(0, KT, 2):
                    for s in range(3):
                        nc.tensor.matmul(
                            out=g[:, COFF[s] : COFF[s] + CSUB[s]],
                            lhsT=ft[:, k : k + 2, nt * 128 : (nt + 1) * 128],
                            rhs=ct_t[:, k : k + 2, COFF[s] : COFF[s] + CSUB[s]],
                            start=(k == 0),
                            stop=False,
                            perf_mode=mybir.MatmulPerfMode.DoubleRow,
                        )
                for s in range(3):
                    nc.tensor.matmul(
                        out=g[:, COFF[s] : COFF[s] + CSUB[s]],
                        lhsT=ones_b,
                        rhs=csqrow[:1, COFF[s] : COFF[s] + CSUB[s]],
                        start=False,
                        stop=True,
                    )
                scr_e = expp.tile([128, CPC], BF16, tag="scr_e")
                nc.scalar.activation(
                    scr_e,
                    g[:, :CPC],
                    mybir.ActivationFunctionType.Exp,
                    scale=2.0 / (FS * CS),
                    accum_out=accg[:, nt : nt + 1],
                )

        # ---- label path (independent; gather from full centers in DRAM) ----
        for nt in range(NTL):
            labt = small.tile([128, 1], I32, tag="labt")
            d1 = nc.sync.dma_start(
                out=labt, in_=lab.ap()[nt * 128 : (nt + 1) * 128, :]
            )
            tile.add_dep_helper(d1.ins, ft_dmas[3].ins, True, "defer label path")
            crows = small.tile([128, D], F32, tag="crows")
            nc.gpsimd.indirect_dma_start(
                out=crows,
                out_offset=None,
                in_=cfull.ap(),
                in_offset=bass.IndirectOffsetOnAxis(ap=labt[:, :1], axis=0),
            )
            fnt = small.tile([128, D], F32, tag="fnt")
            d2 = nc.sync.dma_start(
                out=fnt, in_=fnat.ap()[nt * 128 : (nt + 1) * 128, :]
            )
            tile.add_dep_helper(d2.ins, ft_dmas[3].ins, True, "defer label path")
            scr1 = scrp.tile([128, D], F32, tag="lab_scr")
            _ttr(nc, scr1, fnt, crows, cl4[:, nt : nt + 1], 0.0)
            scr2 = scrp.tile([128, D], F32, tag="lab_scr")
            _ttr(nc, scr2, crows, crows, cq4[:, nt : nt + 1], 0.0)
            scr3 = scrp.tile([128, D], F32, tag="lab_scr")
            _ttr(nc, scr3, fnt, fnt, fsq4[:, nt : nt + 1], 0.0)

        # ---- combine partial sumexp across cores (DEBUG: host-side) ----
        nc.sync.dma_start(out=out2.ap(), in_=accg)

        # ---- finals ----
        nc.vector.memset(fin3[:, 0:1], 0.0)
        glab4 = small.tile([128, NTL], F32, tag="glab4")
        nc.vector.tensor_scalar_mul(glab4, cl4, 2.0)
        nc.vector.tensor_sub(glab4, glab4, cq4)
        nc.vector.reduce_sum(fin3[:, 1:2], glab4, axis=mybir.AxisListType.X)
        nc.vector.reduce_sum(fin3[:, 2:3], fsq4, axis=mybir.AxisListType.X)
        fin_ps = ps_small.tile([1, 3], F32, tag="fin_ps")
        nc.tensor.matmul(out=fin_ps, lhsT=ones_f, rhs=fin3, start=True, stop=True)
        out_sb = small.tile([1, 3], F32, tag="out_sb")
        nc.scalar.copy(out_sb, fin_ps)
        nc.sync.dma_start(out=out.ap(), in_=out_sb)

    nc.compile()
    return nc


def _get_nc():
    if "nc" not in _CACHE:
        _CACHE["nc"] = _build()
    return _CACHE["nc"]


def make_in_maps(feat, label, centers):
    feat = np.ascontiguousarray(np.asarray(feat, dtype=np.float32))
    centers = np.ascontiguousarray(np.asarray(centers, dtype=np.float32))
    label = np.ascontiguousarray(np.asarray(label).astype(np.int32).reshape(N, 1))

    bf = ml_dtypes.bfloat16
    f8 = ml_dtypes.float8_e4m3
    cT_pad = np.zeros((D, CP), dtype=f8)
    cT_pad[:, :C] = (centers.T * CS).astype(f8)
    cnat_pad = np.full((CP, D), PADV, dtype=bf)
    cnat_pad[:C, :] = centers.astype(bf)
    featT = np.ascontiguousarray((feat.T * FS).astype(f8))  # [D, N]

    in_maps = []
    for i in range(NCORES):
        sl = slice(i * NPC, (i + 1) * NPC)
        cs = slice(i * CPC, (i + 1) * CPC)
        in_maps.append(
            {
                "ftt": featT,
                "fnat": np.ascontiguousarray(feat[sl]),
                "lab": np.ascontiguousarray(label[sl]),
                "ct": np.ascontiguousarray(cT_pad[:, cs]),
                "cnat": np.ascontiguousarray(cnat_pad[cs]),
                "cfull": centers,
            }
        )
    return in_maps


def combine(parts, accgs):
    parts = np.asarray(parts, dtype=np.float64)
    sumexp = np.zeros((128, NT), dtype=np.float64)
    for a in accgs:
        sumexp += np.asarray(a, dtype=np.float64)
    lse_sum = float(np.log(sumexp).sum())  # DEBUG ONLY: host-side ln
    glab_sum = parts[:, 1].sum()
    s1 = parts[:, 2].sum()
    nll_sum = lse_sum - glab_sum
    centerloss = (s1 - glab_sum) / (2.0 * N)
    ddaloss = nll_sum / (2.0 * N * N)
    loss = LAMB * centerloss + GAMMA * ddaloss
    return loss, centerloss, ddaloss


def kernel(feat, label, centers):
    from concourse.bass_utils import run_bass_kernel_spmd

    in_maps = make_in_maps(feat, label, centers)
    nc = _get_nc()
    res = run_bass_kernel_spmd(nc, in_maps, core_ids=list(range(NCORES)))
    parts = [r["out"].reshape(3) for r in res.results]
    accgs = [r["out2"] for r in res.results]
    loss, centerloss, ddaloss = combine(parts, accgs)
    return (
        np.float32(loss),
        np.float32(centerloss),
        np.float32(ddaloss),
    )


# revision 15
# speedup vs baseline: 1.7144x; 1.6796x over previous
"""DDALoss Trainium2 kernel (8 NeuronCores, class-sharded softmax).

Math (algebraically identical to the reference):
  g[n,c]     = 2*feat[n]@centers[c] - ||centers[c]||^2          (logits shifted
               by the row-constant ||feat[n]||^2, which cancels in softmax)
  lse[n]     = log(sum_c exp(g[n,c]))
  glab[n]    = g[n, label[n]]
  nll_sum    = sum_n (lse[n] - glab[n])
  S1         = sum(feat^2)
  centerloss = (S1 - sum_n glab[n]) / (2N)
  ddaloss    = nll_sum / (2N^2)
  loss       = LAMB*centerloss + GAMMA*ddaloss

Sharding: classes are split 8 ways (1280 padded classes per core); every core
sees all 4096 batch rows.  vs batch-sharding this cuts per-core DMA from
~18MB to ~6MB (no full 10240x512 bf16 centers stream per core) and the csq
DVE work by 8x.  Per-row partial sums of exp are combined with a 16KB
AllReduce; the label/gather path stays batch-sharded (rows i*512..(i+1)*512
on core i) so scalar partials just sum on the host.

Per-core schedule:
  - everything SBUF-resident up front: featT fp8 [128,4,4096] (weights),
    centers-shard^T fp8 [128,4,1280] (moving), centers-shard bf16 natural
    [128,10,512] (for csq only).
  - csq chain: 10 DVE TENSOR_TENSOR_REDUCE squares -> csqn[:,j], PE transpose
    -> fp8 cast -> DRAM roundtrip -> csqrow [1,2,1280] fp8 (plane 1 zeros).
  - main loop over 32 batch tiles: psum[n128, c1280] accumulates 6 fp8
    DoubleRow matmuls (K=512) plus 3 K=2 DoubleRow "ones x (-csq*FS*CS/2)"
    bias matmuls; ACT exp(scale*psum) with accum_out -> partial sumexp col.
  - AllReduce [128,32] partial sumexp across the 8 cores, ln -> lse, then
    sum-reduce. Every core emits the identical full lse_sum (host divides
    by NCORES); glab/S1 partials are per-core as in the batch-sharded path.
  - output: [1,3] partials (lse_sum, glab_sum, S1); final combine on host.
"""

import sys

sys.path.insert(0, "/opt/trn_rl_repo")

import numpy as np
import ml_dtypes

from contextlib import ExitStack

import concourse.bass as bass
import concourse.bacc as bacc
import concourse.tile as tile
from concourse import mybir

# Problem constants (hardcoded per harness contract)
N = 4096
D = 512
C = 10000
CP = 10240  # classes padded to 128*80
NCORES = 8
CPC = CP // NCORES  # 1280 classes per core
NPC = N // NCORES  # 512 label rows per core
NT = N // 128  # 32 batch tiles per core (all rows)
NTL = NPC // 128  # 4 label tiles per core
KT = D // 128  # 4 contraction blocks
CSUB = [512, 512, 256]  # class sub-chunks within the 1280-wide psum tile
COFF = [0, 512, 1024]

LAMB = 0.01
GAMMA = 3.0

BF16 = mybir.dt.bfloat16
FP8 = mybir.dt.float8e4
F32 = mybir.dt.float32
I32 = mybir.dt.int32

# fp8 scaling: feat*FS and centers*CS on host keep e4m3 values in the normal
# range; psum holds FS*CS*cross and ACT's exp scale of 2/(FS*CS) restores
# exp(2*cross).  The -csq bias is applied POST-exp: the row sum is the
# DVE TTR-weighted sum  sum_c exp(2cross)*w_c  with w_c = exp(-csq_c),
# which keeps the bias pass off the PE entirely.
FS = 8.0
CS = 16.0
# padded center rows: csq_pad = 512*PADV^2 = 90 makes w_pad = exp(-90) = 0
# in bf16, so pad classes drop out of the weighted row sums exactly.
PADV = 0.42

_CACHE = {}


def _ttr(nc, out, in0, in1, accum_out, init, scale=1.0):
    """accum_out = init + sum_free(in0 * in1 * scale); out = elementwise scratch."""
    from concourse.dve_ops import TENSOR_TENSOR_REDUCE

    nc.vector._custom_dve(
        TENSOR_TENSOR_REDUCE,
        out=out,
        in0=in0,
        in1=in1,
        s0=init,
        s1=scale,
        accum_out=accum_out,
    )


def _build():
    nc = bacc.Bacc(
        "TRN2", target_bir_lowering=False, debug=False, num_devices=NCORES
    )

    # Per-core external inputs
    ftT = nc.dram_tensor("ftt", [D, N], FP8, kind="ExternalInput")  # full feat^T
    fnat = nc.dram_tensor("fnat", [NPC, D], F32, kind="ExternalInput")  # feat rows
    lab = nc.dram_tensor("lab", [NPC, 1], I32, kind="ExternalInput")
    cT = nc.dram_tensor("ct", [D, CPC], FP8, kind="ExternalInput")  # centers shard^T
    cnat = nc.dram_tensor("cnat", [CPC, D], BF16, kind="ExternalInput")  # shard natural
    cfull = nc.dram_tensor("cfull", [C, D], F32, kind="ExternalInput")  # for gather
    out = nc.dram_tensor("out", [1, 3], F32, kind="ExternalOutput")
    out2 = nc.dram_tensor("out2", [128, NT], F32, kind="ExternalOutput")
    csq_dram = nc.dram_tensor("csq_scratch", [CPC // 128, 128], BF16, kind="Internal")

    with tile.TileContext(nc) as tc, ExitStack() as ctx:
        const = ctx.enter_context(tc.tile_pool(name="const", bufs=1))
        small = ctx.enter_context(tc.tile_pool(name="small", bufs=2))
        scrp = ctx.enter_context(tc.tile_pool(name="scrp", bufs=2))
        expp = ctx.enter_context(tc.tile_pool(name="expp", bufs=4))
        ttrp = ctx.enter_context(tc.tile_pool(name="ttrp", bufs=2))
        ps_small = ctx.enter_context(tc.tile_pool(name="ps_small", bufs=1, space="PSUM"))

        # ---- constants / persistent tiles ----
        ones_f = const.tile([128, 1], F32)
        nc.vector.memset(ones_f, 1.0)
        ones_b = const.tile([1, 128], BF16)
        nc.vector.memset(ones_b, 1.0)
        ident = const.tile([128, 128], F32, tag="ident")
        from concourse.masks import make_identity

        make_identity(nc, ident)

        # all SBUF-resident operands, loaded once (cnat first: the csq/w
        # chain has the longest dependency tail before the main loop)
        cn = const.tile([128, CPC // 128, D], BF16, tag="cn")
        cnat_r = cnat.ap().rearrange("(x p) d -> p x d", p=128)
        nc.sync.dma_start(out=cn, in_=cnat_r)

        ct_t = const.tile([128, KT, CPC], FP8, tag="ct_t")
        cT_r = cT.ap().rearrange("(k p) c -> p k c", p=128)
        nc.sync.dma_start(out=ct_t, in_=cT_r)

        ft = const.tile([128, KT, N], FP8, tag="ft")
        ftT_r = ftT.ap().rearrange("(k p) n -> p k n", p=128)
        ft_dmas = []
        for i in range(4):
            ft_dmas.append(
                nc.sync.dma_start(
                    out=ft[:, :, i * 1024 : (i + 1) * 1024],
                    in_=ftT_r[:, :, i * 1024 : (i + 1) * 1024],
                )
            )

        csqn = const.tile([128, CPC // 128], F32, tag="csqn")  # -csq
        csqrow = const.tile([1, CPC], BF16, tag="csqrow")  # -csq row, class order
        wb = const.tile([128, CPC], BF16, tag="wb")  # exp(-csq) broadcast
        accg = const.tile([128, NT], F32, tag="accg")  # per-nt partial sumexp
        cl4 = const.tile([128, NTL], F32, tag="cl4")
        cq4 = const.tile([128, NTL], F32, tag="cq4")
        fsq4 = const.tile([128, NTL], F32, tag="fsq4")
        fin3 = const.tile([128, 3], F32, tag="fin3")

        # ---- csq chain: cn -> csqn -> (transpose+cast) -> csqrow -> wb ----
        for j in range(CPC // 128):
            scr = scrp.tile([128, D], BF16, tag="csq_scr")
            _ttr(
                nc,
                scr,
                cn[:, j, :],
                cn[:, j, :],
                csqn[:, j : j + 1],
                0.0,
                scale=-1.0,
            )
        tp = ps_small.tile([CPC // 128, 128], F32, tag="tp")
        nc.tensor.transpose(out=tp, in_=csqn[:, :], identity=ident)
        tp_b = small.tile([CPC // 128, 128], BF16, tag="tp_b")
        nc.vector.tensor_copy(tp_b, tp)
        nc.sync.dma_start(out=csq_dram.ap(), in_=tp_b)
        nc.sync.dma_start(
            out=csqrow,
            in_=bass.AP(tensor=csq_dram, offset=0, ap=[[0, 1], [1, CPC]]),
        )

        # ---- main loop over batch tiles ----
        with tc.tile_pool(name="ps_g", bufs=2, space="PSUM") as ps_g:
            # wb = exp(-csq) broadcast to all partitions (ones x csqrow, exp)
            wps = ps_g.tile([128, 1536], F32, tag="g")
            for s in range(3):
                nc.tensor.matmul(
                    out=wps[:, COFF[s] : COFF[s] + CSUB[s]],
                    lhsT=ones_b,
                    rhs=csqrow[:1, COFF[s] : COFF[s] + CSUB[s]],
                    start=True,
                    stop=True,
                )
            nc.scalar.activation(wb, wps[:, :CPC], mybir.ActivationFunctionType.Exp)

            for nt in range(NT):
                g = ps_g.tile([128, 1536], F32, tag="g")
                for k in range(0, KT, 2):
                    for s in range(3):
                        nc.tensor.matmul(
                            out=g[:, COFF[s] : COFF[s] + CSUB[s]],
                            lhsT=ft[:, k : k + 2, nt * 128 : (nt + 1) * 128],
                            rhs=ct_t[:, k : k + 2, COFF[s] : COFF[s] + CSUB[s]],
                            start=(k == 0),
                            stop=(k == 2),
                            perf_mode=mybir.MatmulPerfMode.DoubleRow,
                        )
                scr_e = expp.tile([128, CPC], BF16, tag="scr_e")
                nc.scalar.activation(
                    scr_e,
                    g[:, :CPC],
                    mybir.ActivationFunctionType.Exp,
                    scale=2.0 / (FS * CS),
                )
                scr_t = ttrp.tile([128, CPC], BF16, tag="scr_t")
                _ttr(nc, scr_t, scr_e, wb, accg[:, nt : nt + 1], 0.0)

        # ---- label path (independent; gather from full centers in DRAM) ----
        for nt in range(NTL):
            labt = small.tile([128, 1], I32, tag="labt")
            d1 = nc.sync.dma_start(
                out=labt, in_=lab.ap()[nt * 128 : (nt + 1) * 128, :]
            )
            tile.add_dep_helper(d1.ins, ft_dmas[3].ins, True, "defer label path")
            crows = small.tile([128, D], F32, tag="crows")
            nc.gpsimd.indirect_dma_start(
                out=crows,
                out_offset=None,
                in_=cfull.ap(),
                in_offset=bass.IndirectOffsetOnAxis(ap=labt[:, :1], axis=0),
            )
            fnt = small.tile([128, D], F32, tag="fnt")
            d2 = nc.sync.dma_start(
                out=fnt, in_=fnat.ap()[nt * 128 : (nt + 1) * 128, :]
            )
            tile.add_dep_helper(d2.ins, ft_dmas[3].ins, True, "defer label path")
            scr1 = scrp.tile([128, D], F32, tag="lab_scr")
            _ttr(nc, scr1, fnt, crows, cl4[:, nt : nt + 1], 0.0)
            scr2 = scrp.tile([128, D], F32, tag="lab_scr")
            _ttr(nc, scr2, crows, crows, cq4[:, nt : nt + 1], 0.0)
            scr3 = scrp.tile([128, D], F32, tag="lab_scr")
            _ttr(nc, scr3, fnt, fnt, fsq4[:, nt : nt + 1], 0.0)

        # ---- combine partial sumexp across cores (DEBUG: host-side) ----
        nc.sync.dma_start(out=out2.ap(), in_=accg)

        # ---- finals ----
        nc.vector.memset(fin3[:, 0:1], 0.0)
        glab4 = small.tile([128, NTL], F32, tag="glab4")
        nc.vector.tensor_scalar_mul(glab4, cl4, 2.0)
        nc.vector.tensor_sub(glab4, glab4, cq4)
        nc.vector.reduce_sum(fin3[:, 1:2], glab4, axis=mybir.AxisListType.X)
        nc.vector.reduce_sum(fin3[:, 2:3], fsq4, axis=mybir.AxisListType.X)
        fin_ps = ps_small.tile([1, 3], F32, tag="fin_ps")
        nc.tensor.matmul(out=fin_ps, lhsT=ones_f, rhs=fin3, start=True, stop=True)
        out_sb = small.tile([1, 3], F32, tag="out_sb")
        nc.scalar.copy(out_sb, fin_ps)
        nc.sync.dma_start(out=out.ap(), in_=out_sb)

    nc.compile()
    return nc


def _get_nc():
    if "nc" not in _CACHE:
        _CACHE["nc"] = _build()
    return _CACHE["nc"]


def make_in_maps(feat, label, centers):
    feat = np.ascontiguousarray(np.asarray(feat, dtype=np.float32))
    centers = np.ascontiguousarray(np.asarray(centers, dtype=np.float32))
    label = np.ascontiguousarray(np.asarray(label).astype(np.int32).reshape(N, 1))

    bf = ml_dtypes.bfloat16
    f8 = ml_dtypes.float8_e4m3
    cT_pad = np.zeros((D, CP), dtype=f8)
    cT_pad[:, :C] = (centers.T * CS).astype(f8)
    cnat_pad = np.full((CP, D), PADV, dtype=bf)
    cnat_pad[:C, :] = centers.astype(bf)
    featT = np.ascontiguousarray((feat.T * FS).astype(f8))  # [D, N]

    in_maps = []
    for i in range(NCORES):
        sl = slice(i * NPC, (i + 1) * NPC)
        cs = slice(i * CPC, (i + 1) * CPC)
        in_maps.append(
            {
                "ftt": featT,
                "fnat": np.ascontiguousarray(feat[sl]),
                "lab": np.ascontiguousarray(label[sl]),
                "ct": np.ascontiguousarray(cT_pad[:, cs]),
                "cnat": np.ascontiguousarray(cnat_pad[cs]),
                "cfull": centers,
            }
        )
    return in_maps


def combine(parts, accgs):
    parts = np.asarray(parts, dtype=np.float64)
    sumexp = np.zeros((128, NT), dtype=np.float64)
    for a in accgs:
        sumexp += np.asarray(a, dtype=np.float64)
    lse_sum = float(np.log(sumexp).sum())  # DEBUG ONLY: host-side ln
    glab_sum = parts[:, 1].sum()
    s1 = parts[:, 2].sum()
    nll_sum = lse_sum - glab_sum
    centerloss = (s1 - glab_sum) / (2.0 * N)
    ddaloss = nll_sum / (2.0 * N * N)
    loss = LAMB * centerloss + GAMMA * ddaloss
    return loss, centerloss, ddaloss


def kernel(feat, label, centers):
    from concourse.bass_utils import run_bass_kernel_spmd

    in_maps = make_in_maps(feat, label, centers)
    nc = _get_nc()
    res = run_bass_kernel_spmd(nc, in_maps, core_ids=list(range(NCORES)))
    parts = [r["out"].reshape(3) for r in res.results]
    accgs = [r["out2"] for r in res.results]
    loss, centerloss, ddaloss = combine(parts, accgs)
    return (
        np.float32(loss),
        np.float32(centerloss),
        np.float32(ddaloss),
    )
